# revision 15
# baseline (speedup 1.0000x reference)
"""BartLatentAttention Trainium2 kernel.

Full-input contract: kernel(**inputs) takes the unsharded tensors from
setup_inputs() and returns the full [B, T, D] float32 output.

Sharding: tensor-parallel over heads. 16 heads / 8 cores = 2 heads per
core. Each core computes q/k/v projections for its 2 heads (column-sliced
weights), attention over the latent-prefixed sequence, and a partial
output projection (row-sliced Wo). The host sums the 8 partial outputs
and adds bo.

Device-side layout notes:
  - hidden is fed pre-tiled as hti [NTC, 128, NKC, 512] bf16 so each
    512-token group loads with one 8KB-per-partition contiguous DMA.
  - scores are computed transposed (scoresT [s, t]) so that softmax's
    exp can run on ScalarE straight out of PSUM, and the AV matmul can
    consume expT as the moving operand with V [s, d] stationary.
  - V carries an extra ones-column (M=65): PSUM row 64 of the AV
    accumulation is the softmax denominator Z for free.
  - S = L + T = 2056 is laid out padded to 2176 = 17*128: chunk 0 holds
    the 8 latent positions + 120 dead rows (killed with an exp bias of
    -30), chunks 1..16 hold the 2048 token positions.
  - epilogue (1/Z broadcast via a tiny fp32r matmul, out-projection in
    bf16); the partial output is written bf16 and summed on host.
  - schedule: attention chunks are emitted as soon as their kv group is
    projected; remaining projection/epilogue work is paced into the
    exp-bound chunk pipeline via explicit closure queues. Emission order
    defines Tile dependencies, so each chunk force-drains the projection
    closures for the kv groups it reads.
"""

import sys

if "/opt/trn_rl_repo" not in sys.path:
    sys.path.insert(0, "/opt/trn_rl_repo")

import numpy as np
import ml_dtypes

BF16 = ml_dtypes.bfloat16

B, T, D = 2, 2048, 1024
H = 16
HD = D // H  # 64
L = 8
S = L + T  # 2056
SCALE = HD ** -0.5
NCORES = 8
HPC = H // NCORES  # heads per core = 2
DC = HPC * HD  # per-core feature width = 128

BT = B * T  # 4096
NKC = D // 128  # k chunks = 8
NTC = BT // 512  # token chunks of 512 = 8
SCHUNKS = 17  # padded S = 2176 = 17 * 128
TB = 512  # attention t-block
NTB = T // TB  # 4 per batch
PAD_BIAS = -30.0

_cache: dict = {}


def _build_nc():
    import concourse.bass as bass
    import concourse.mybir as mybir
    import concourse.tile as tile
    from concourse import bacc

    fp32 = mybir.dt.float32
    fp32r = mybir.dt.float32r
    bf16 = mybir.dt.bfloat16

    nc = bacc.Bacc(
        "TRN2",
        target_bir_lowering=False,
        debug=False,
        enable_asserts=False,
        num_devices=NCORES,
    )

    # DRAM I/O (host-retiled for contiguous per-partition DMA lines)
    hti = nc.dram_tensor("hti", [NTC, 128, NKC, 512], bf16,
                         kind="ExternalInput").ap()
    wq = nc.dram_tensor("wq", [128, NKC, DC], bf16, kind="ExternalInput").ap()
    wk = nc.dram_tensor("wk", [128, NKC, DC], bf16, kind="ExternalInput").ap()
    wv = nc.dram_tensor("wv", [128, NKC, DC], bf16, kind="ExternalInput").ap()
    bq = nc.dram_tensor("bq", [DC, 1], fp32, kind="ExternalInput").ap()
    bk = nc.dram_tensor("bk", [DC, 1], fp32, kind="ExternalInput").ap()
    bv1 = nc.dram_tensor("bv1", [DC, 1], fp32, kind="ExternalInput").ap()
    wo = nc.dram_tensor("wo", [DC, D], bf16, kind="ExternalInput").ap()
    lkT = nc.dram_tensor("lkT", [B, DC, L], bf16, kind="ExternalInput").ap()
    lv = nc.dram_tensor("lv", [B, HPC, L, HD], bf16, kind="ExternalInput").ap()
    ebias0 = nc.dram_tensor("ebias0", [128, 1], fp32, kind="ExternalInput").ap()
    e2 = nc.dram_tensor("e2", [2, 128], fp32, kind="ExternalInput").ap()
    out = nc.dram_tensor("out", [BT, D], bf16, kind="ExternalOutput").ap()

    EXP = mybir.ActivationFunctionType.Exp

    with tile.TileContext(nc) as tc:
        with (
            tc.tile_pool(name="consts", bufs=1) as consts,
            tc.tile_pool(name="persist", bufs=1) as persist,
            tc.tile_pool(name="htiles", bufs=3) as htiles,
            tc.tile_pool(name="exps", bufs=4) as exps,
            tc.tile_pool(name="episb", bufs=2) as episb,
        ):
            # ---- constants (wq + first hidden group emitted first so the
            # q projection can start as early as possible) ----
            wq_sb = consts.tile([128, NKC, DC], bf16)
            wk_sb = consts.tile([128, NKC, DC], bf16)
            wv_sb = consts.tile([128, NKC, DC], bf16)
            nc.sync.dma_start(out=wq_sb, in_=wq)
            nc.gpsimd.dma_start(out=wk_sb, in_=wk)
            nc.gpsimd.dma_start(out=wv_sb, in_=wv)
            bq_sb = consts.tile([DC, 1], fp32)
            bk_sb = consts.tile([DC, 1], fp32)
            bv1_sb = consts.tile([DC, 1], fp32)
            wo_sb = consts.tile([DC, D], bf16)
            eb0_sb = consts.tile([128, 1], fp32)
            e2_sb = consts.tile([2, 128], fp32)
            e2_r = consts.tile([2, 128], fp32r)
            nc.sync.dma_start(out=bq_sb, in_=bq)
            nc.sync.dma_start(out=bk_sb, in_=bk)
            nc.sync.dma_start(out=bv1_sb, in_=bv1)
            nc.sync.dma_start(out=wo_sb, in_=wo)
            nc.sync.dma_start(out=e2_sb, in_=e2)
            nc.vector.tensor_copy(e2_r, e2_sb)
            nc.sync.dma_start(out=eb0_sb, in_=ebias0)

            # ---- persistent activations ----
            qT_sb = persist.tile([128, BT], bf16)  # [h0|h1 feats, global tok]
            kT_sb = persist.tile([128, B * SCHUNKS * 128], bf16)  # per b: 2176
            v_sb = persist.tile([128, B * HPC * SCHUNKS * 65], bf16)

            def k_off(b):
                return b * SCHUNKS * 128

            def v_off(b, h, c):
                return ((b * HPC + h) * SCHUNKS + c) * 65

            # latent / pad setup
            for b in range(B):
                nc.vector.memset(kT_sb[:, k_off(b) + L:k_off(b) + 128], 0.0)
                nc.sync.dma_start(out=kT_sb[:, k_off(b):k_off(b) + L],
                                  in_=lkT[b])
                for h in range(HPC):
                    nc.vector.memset(
                        v_sb[:, v_off(b, h, 0):v_off(b, h, 0) + 65], 0.0)
            # ones column for the Z fold (col 64 of every [128, 65] chunk)
            v_view = v_sb.rearrange("p (n c) -> p n c", c=65)
            nc.vector.memset(v_view[:, :, 64:65], 1.0)
            for b in range(B):
                for h in range(HPC):
                    nc.sync.dma_start(
                        out=v_sb[0:L, v_off(b, h, 0):v_off(b, h, 0) + HD],
                        in_=lv[b, h])

            # identity for PE transposes
            ident = consts.tile([128, 128], bf16)
            from concourse.masks import make_identity
            make_identity(nc, ident)

            # ---- psum pools (shared by qkv passes and attention) ----
            with (
                tc.tile_pool(name="scps", bufs=2, space="PSUM") as scps,
                tc.tile_pool(name="avps", bufs=4, space="PSUM") as avps,
            ):
                # dummy matmuls during the startup DMA wait: hold the PE
                # busy so HAM unthrottles before the first real matmul
                warm = avps.tile([128, 128], fp32, tag="av", name="warm")
                for _ in range(90):
                    nc.tensor.matmul(warm, ident, ident,
                                     start=True, stop=True)
                # ---- qkv projection closures for one 512-token chunk ----
                def qkv_closures(g):
                    t0g = g * 512
                    bb = t0g // T
                    c0 = (t0g - bb * T) // 128 + 1
                    hold = {}

                    def ht_a():
                        ht = htiles.tile([128, NKC, 512], bf16, tag="ht",
                                         name=f"ht_{g}")
                        nc.gpsimd.dma_start(out=ht[:, 0:4, :],
                                            in_=hti[g, :, 0:4, :])
                        hold["ht"] = ht

                    def ht_b():
                        nc.sync.dma_start(out=hold["ht"][:, 4:NKC, :],
                                          in_=hti[g, :, 4:NKC, :])

                    def mk_proj(key, w_sb):
                        def mk_part(k0, k1, tag=None):
                            def part():
                                if k0 == 0:
                                    ps = avps.tile([128, 512], fp32, tag="av",
                                                   name=f"{key}ps_{g}")
                                    hold[key] = ps
                                ps = hold[key]
                                for k in range(k0, k1):
                                    nc.tensor.matmul(
                                        ps, w_sb[:, k, :],
                                        hold["ht"][:, k, :],
                                        start=(k == 0), stop=(k == NKC - 1))
                            return part
                        return [mk_part(0, 2), mk_part(2, 4),
                                mk_part(4, 6), mk_part(6, 8)]

                    qp = mk_proj("q", wq_sb)
                    kp = mk_proj("k", wk_sb)
                    vp = mk_proj("v", wv_sb)

                    def q_fin():
                        nc.vector.tensor_scalar_add(
                            qT_sb[:, t0g:t0g + 512], hold["q"], bq_sb)

                    def k_fin():
                        koff = k_off(bb) + 128 + (t0g - bb * T)
                        nc.vector.tensor_scalar_add(
                            kT_sb[:, koff:koff + 512], hold["k"], bk_sb)

                    def v_fin():
                        vt = episb.tile([128, 512], bf16, tag="vt",
                                        name=f"vt_{g}")
                        nc.vector.tensor_scalar_add(vt, hold["v"], bv1_sb)
                        hold["vt"] = vt

                    def t_a():
                        tp = avps.tile([128, 512], bf16, tag="av",
                                       name=f"tp_{g}")
                        hold["tp"] = tp
                        for j in range(2):
                            nc.tensor.transpose(
                                tp[:, j * 128:(j + 1) * 128],
                                hold["vt"][:, j * 128:(j + 1) * 128], ident)

                    def t_b():
                        tp = hold["tp"]
                        for j in range(2, 4):
                            nc.tensor.transpose(
                                tp[:, j * 128:(j + 1) * 128],
                                hold["vt"][:, j * 128:(j + 1) * 128], ident)
                        # v_sb[:, (c0+m, h, d)] = tp[:, (m, h, d)]
                        dst = bass.AP(
                            tensor=v_sb.tensor,
                            offset=v_sb.offset + v_off(bb, 0, c0),
                            ap=[v_sb.ap[0], [65, 4], [SCHUNKS * 65, HPC],
                                [1, HD]])
                        srcv = tp.rearrange("p (m e) -> p m e", m=4)
                        src = bass.AP(
                            tensor=srcv.tensor, offset=srcv.offset,
                            ap=[srcv.ap[0], [128, 4], [64, HPC], [1, HD]])
                        nc.vector.tensor_copy(dst, src)

                    return ([ht_a, ht_b] + qp + [q_fin] + kp + [k_fin]
                            + vp + [v_fin, t_a, t_b])

                # ---- attention helpers ----
                def emit_epi_drain(st):
                    av0, av1 = st["av0"], st["av1"]
                    oz = episb.tile([128, 512], fp32, tag="oz",
                                    name=f"oz_{st['q0']}")
                    zh0 = episb.tile([1, 512], fp32, tag="zh0",
                                     name=f"zh0_{st['q0']}")
                    zh1 = episb.tile([1, 512], fp32, tag="zh1",
                                     name=f"zh1_{st['q0']}")
                    zr2 = episb.tile([2, 512], fp32, tag="zr2",
                                     name=f"zr2_{st['q0']}")
                    nc.vector.tensor_copy(zh0, av0[64:65, :])
                    nc.vector.tensor_copy(zh1, av1[64:65, :])
                    nc.gpsimd.dma_start(out=zr2[0:1, :], in_=zh0)
                    nc.gpsimd.dma_start(out=zr2[1:2, :], in_=zh1)
                    nc.vector.tensor_copy(oz[0:64, :], av0[0:64, :])
                    nc.vector.tensor_copy(oz[64:128, :], av1[0:64, :])
                    nc.vector.reciprocal_approx_fast(out=zr2, in_=zr2)
                    st["oz"], st["zr2"] = oz, zr2

                def emit_epi_zb(st):
                    zr2r = episb.tile([2, 512], fp32r, tag="zr2r",
                                      name=f"zr2r_{st['q0']}")
                    nc.vector.tensor_copy(zr2r, st["zr2"])
                    zb = avps.tile([128, 512], fp32, tag="av",
                                   name=f"zb_{st['q0']}")
                    nc.tensor.matmul(zb, e2_r, zr2r,
                                     start=True, stop=True)
                    ot = episb.tile([128, 512], bf16, tag="ot",
                                    name=f"ot_{st['q0']}")
                    nc.vector.tensor_mul(ot, st["oz"], zb)
                    st["ot"] = ot

                def mk_epi_out(st, j):
                    # out-proj for one 128-token block: 2 bf16 matmuls into
                    # two psum banks, one [128, 1024] bf16 copy, one DMA
                    def mm():
                        ot, q0 = st["ot"], st["q0"]
                        ops = []
                        for f in range(2):
                            op = avps.tile([128, 512], fp32, tag="av",
                                           name=f"op_{q0}_{j}_{f}")
                            nc.tensor.matmul(
                                op, ot[:, j * 128:(j + 1) * 128],
                                wo_sb[:, f * 512:(f + 1) * 512],
                                start=True, stop=True)
                            ops.append(op)
                        st[f"op{j}"] = ops

                    def wr():
                        q0 = st["q0"]
                        ops = st.pop(f"op{j}")
                        osb = episb.tile([128, D], bf16, tag="osb",
                                         name=f"osb_{q0}_{j}")
                        nc.vector.tensor_copy(osb[:, 0:512], ops[0])
                        nc.vector.tensor_copy(osb[:, 512:1024], ops[1])
                        r0 = q0 + j * 128
                        nc.sync.dma_start(out=out[r0:r0 + 128, :], in_=osb)
                    return [mm, wr]

                def emit_av(st, c):
                    b = st["b"]
                    stt, sp = c == 0, c == SCHUNKS - 1
                    ex = st["ex"].pop(c)
                    for h, av in ((0, st["av0"]), (1, st["av1"])):
                        vo = v_off(b, h, c)
                        eh = ex[:, h * 512:(h + 1) * 512]
                        nc.tensor.matmul(
                            av, v_sb[:, vo:vo + 65], eh,
                            start=stt, stop=sp)

                # ---- schedule: g0 inline; everything else paced through
                # two queues. gq holds qkv-group closures (tagged by group)
                # so chunks can force-drain exactly the groups whose k/v
                # they read -- emission order defines Tile dependencies, so
                # a chunk's scores must be emitted after its group's k_fin.
                from collections import deque
                g0 = qkv_closures(0)
                for cl in g0[:7]:  # ht loads, q projection, q_fin
                    cl()
                gq = deque()
                for g in range(1, NTC):
                    for cl in qkv_closures(g):
                        gq.append((g, cl))
                eq = deque()
                for _ in range(2):  # prefetch g1's ht DMAs
                    gq.popleft()[1]()

                def drain_groups(gmax):
                    while gq and gq[0][0] <= gmax:
                        gq.popleft()[1]()

                def pop_side(n):
                    for i in range(n):
                        if eq and (i % 2 == 0 or not gq):
                            eq.popleft()()
                        elif gq:
                            gq.popleft()[1]()
                        elif eq:
                            eq.popleft()()

                def mk_st(b, tb):
                    return {
                        "b": b, "q0": b * T + tb * TB,
                        "av0": avps.tile([65, 512], fp32, tag="av",
                                         name=f"av0_{b}_{tb}"),
                        "av1": avps.tile([65, 512], fp32, tag="av",
                                         name=f"av1_{b}_{tb}"),
                        "ex": {},
                    }

                def emit_chunk(st, c):
                    b, q0 = st["b"], st["q0"]
                    sc = scps.tile([128, 1024], fp32, tag="sc",
                                   name=f"sc_{b}_{q0}_{c}")
                    kc = k_off(b) + c * 128
                    nc.tensor.matmul(
                        sc[:, 0:512],
                        kT_sb[0:64, kc:kc + 128],
                        qT_sb[0:64, q0:q0 + TB],
                        start=True, stop=True)
                    nc.tensor.matmul(
                        sc[:, 512:1024],
                        kT_sb[64:128, kc:kc + 128],
                        qT_sb[64:128, q0:q0 + TB],
                        start=True, stop=True)
                    ex = exps.tile([128, 1024], bf16, tag="ex",
                                   name=f"ex_{b}_{q0}_{c}")
                    nc.scalar.activation(
                        ex, sc, EXP,
                        bias=(eb0_sb if c == 0 else 0.0), scale=1.0)
                    st["ex"][c] = ex

                for b in range(B):
                    for tb in range(NTB):
                        ti = b * NTB + tb
                        st = mk_st(b, tb)
                        if ti == 0:
                            # latent chunk needs only q: overlap its exp
                            # with g0's k/v projections
                            emit_chunk(st, 0)
                            for cl in g0[7:]:
                                cl()
                        else:
                            drain_groups(ti)  # this tb's q projection
                        for c in range(SCHUNKS):
                            if ti == 0 and c == 0:
                                continue
                            if c >= 1:
                                drain_groups(b * NTB + (c - 1) // 4)
                            emit_chunk(st, c)
                            if c >= 1:
                                emit_av(st, c - 1)
                            if tb == 0:
                                # race ahead on this batch's remaining
                                # projections (chunk c+1 group is needed
                                # at most 4 chunks out)
                                if c >= 1:
                                    pop_side(4)
                            elif ti == NTB - 1:
                                # pre-drain b1's projections before the
                                # batch switch
                                if c >= 1:
                                    pop_side(2)
                            else:
                                if c >= 1:
                                    pop_side(1)
                                if c in (5, 9, 13):
                                    pop_side(1)
                        emit_av(st, SCHUNKS - 1)
                        emit_epi_drain(st)
                        # queue this tb's epilogue (runs inside next tb);
                        # two no-op slots let the drain chain finish before zb
                        def mk_zb(s):
                            def go():
                                emit_epi_zb(s)
                            return go
                        noop = lambda: None
                        eq.append(noop)
                        eq.append(noop)
                        eq.append(mk_zb(st))
                        for j in range(4):
                            for cl in mk_epi_out(st, j):
                                eq.append(cl)
                # filler matmuls cover the final drain chain's PE idle
                # window (>3.4us would re-throttle HAM before the last
                # epilogue's matmuls)
                warm2 = avps.tile([128, 512], fp32, tag="av", name="warm2")
                for _ in range(14):
                    nc.tensor.matmul(warm2, ident, qT_sb[:, 0:512],
                                     start=True, stop=True)
                # flush remaining side work (last epilogue + any stragglers)
                while gq:
                    gq.popleft()[1]()
                while eq:
                    eq.popleft()()

    nc.compile()
    return nc


def _get_nc():
    if "nc" not in _cache:
        _cache["nc"] = _build_nc()
    return _cache["nc"]


def _prep_inputs(hidden_states, decoder_latent, Wq, bq, Wk, bk, Wv, bv, Wo):
    """Build the 8 per-core input maps (host-side sharding/layout)."""
    hsD = np.ascontiguousarray(hidden_states.reshape(BT, D))
    # hti[g, p, k, t] = hidden[g*512+t, k*128+p]
    hti = np.ascontiguousarray(
        hsD.reshape(NTC, 512, NKC, 128).transpose(0, 3, 2, 1)).astype(BF16)
    lk = decoder_latent[..., :HD]  # [B, H, L, HD]
    lvf = decoder_latent[..., HD:]
    eb0 = np.full((128, 1), PAD_BIAS, np.float32)
    eb0[:L] = 0.0
    e2 = np.zeros((2, 128), np.float32)
    e2[0, 0:64] = 1.0
    e2[1, 64:128] = 1.0

    def retile_w(w):
        # [D, DC] -> [128, NKC, DC] with [p, k, :] = w[k*128+p, :]
        return np.ascontiguousarray(
            w.reshape(NKC, 128, DC).transpose(1, 0, 2)).astype(BF16)

    in_maps = []
    for c in range(NCORES):
        cols = slice(c * DC, (c + 1) * DC)
        h0, h1 = HPC * c, HPC * c + 1
        lkT_c = np.stack([
            np.concatenate([lk[b, h0].T, lk[b, h1].T], axis=0)
            for b in range(B)])  # [B, 128, L]
        in_maps.append({
            "hti": hti,
            "wq": retile_w(Wq[:, cols] * SCALE),
            "wk": retile_w(Wk[:, cols]),
            "wv": retile_w(Wv[:, cols]),
            "bq": (bq[cols] * SCALE).astype(np.float32).reshape(DC, 1),
            "bk": bk[cols].astype(np.float32).reshape(DC, 1),
            "bv1": bv[cols].astype(np.float32).reshape(DC, 1),
            "wo": Wo[cols, :].astype(BF16),
            "lkT": lkT_c.astype(BF16),
            "lv": lvf[:, h0:h1 + 1].astype(BF16),
            "ebias0": eb0,
            "e2": e2,
        })
    return in_maps


def _run(inputs, trace=False):
    from concourse.bass_utils import run_bass_kernel_spmd

    nc = _get_nc()
    in_maps = _prep_inputs(
        inputs["hidden_states"], inputs["decoder_latent"],
        inputs["Wq"], inputs["bq"], inputs["Wk"], inputs["bk"],
        inputs["Wv"], inputs["bv"], inputs["Wo"])
    res = run_bass_kernel_spmd(nc, in_maps, core_ids=list(range(NCORES)),
                               trace=trace)
    acc = np.zeros((BT, D), np.float64)
    for r in res.results:
        acc += r["out"].astype(np.float64)
    out = (acc + inputs["bo"].astype(np.float64)).astype(np.float32)
    return out.reshape(B, T, D), res


def _reference_fallback(hidden_states, decoder_latent, attention_mask,
                        Wq, bq, Wk, bk, Wv, bv, Wo, bo):
    """Exact numpy path, used only when attention_mask is non-zero (the
    problem spec fills it with zeros; the device kernel specializes on
    that)."""
    x = hidden_states.astype(np.float64)
    q = (x @ Wq + bq) * SCALE
    k = x @ Wk + bk
    v = x @ Wv + bv

    def heads(a):
        return a.reshape(B, T, H, HD).transpose(0, 2, 1, 3)

    q, k, v = heads(q), heads(k), heads(v)
    lk = decoder_latent[..., :HD].astype(np.float64)
    lv = decoder_latent[..., HD:].astype(np.float64)
    k = np.concatenate([lk, k], axis=2)
    v = np.concatenate([lv, v], axis=2)
    s = np.einsum("bhtd,bhsd->bhts", q, k) + attention_mask.astype(np.float64)
    s -= s.max(axis=-1, keepdims=True)
    p = np.exp(s)
    p /= p.sum(axis=-1, keepdims=True)
    o = np.einsum("bhts,bhsd->bhtd", p, v)
    o = o.transpose(0, 2, 1, 3).reshape(B, T, D)
    return (o @ Wo + bo).astype(np.float32)


def kernel(**inputs):
    inputs = {k: np.asarray(v) for k, v in inputs.items()}
    if np.any(inputs["attention_mask"]):
        return _reference_fallback(**inputs)
    out, _ = _run(inputs)
    return out


# revision 16
# speedup vs baseline: 1.0110x; 1.0110x over previous
"""BartLatentAttention Trainium2 kernel.

Full-input contract: kernel(**inputs) takes the unsharded tensors from
setup_inputs() and returns the full [B, T, D] float32 output.

Sharding: tensor-parallel over heads. 16 heads / 8 cores = 2 heads per
core. Each core computes q/k/v projections for its 2 heads (column-sliced
weights), attention over the latent-prefixed sequence, and a partial
output projection (row-sliced Wo). The host sums the 8 partial outputs
and adds bo.

Device-side layout notes:
  - hidden is fed pre-tiled as hti [NTC, 128, NKC, 512] bf16 so each
    512-token group loads with one 8KB-per-partition contiguous DMA.
  - scores are computed transposed (scoresT [s, t]) so that softmax's
    exp can run on ScalarE straight out of PSUM, and the AV matmul can
    consume expT as the moving operand with V [s, d] stationary.
  - V carries an extra ones-column (M=65): PSUM row 64 of the AV
    accumulation is the softmax denominator Z for free.
  - S = L + T = 2056 is laid out padded to 2176 = 17*128: chunk 0 holds
    the 8 latent positions + 120 dead rows (killed with an exp bias of
    -30), chunks 1..16 hold the 2048 token positions.
  - epilogue (1/Z broadcast via a tiny fp32r matmul, out-projection in
    bf16); the partial output is written bf16 and summed on host.
  - schedule: attention chunks are emitted as soon as their kv group is
    projected; remaining projection/epilogue work is paced into the
    exp-bound chunk pipeline via explicit closure queues. Emission order
    defines Tile dependencies, so each chunk force-drains the projection
    closures for the kv groups it reads.
"""

import sys

if "/opt/trn_rl_repo" not in sys.path:
    sys.path.insert(0, "/opt/trn_rl_repo")

import numpy as np
import ml_dtypes

BF16 = ml_dtypes.bfloat16

B, T, D = 2, 2048, 1024
H = 16
HD = D // H  # 64
L = 8
S = L + T  # 2056
SCALE = HD ** -0.5
NCORES = 8
HPC = H // NCORES  # heads per core = 2
DC = HPC * HD  # per-core feature width = 128

BT = B * T  # 4096
NKC = D // 128  # k chunks = 8
NTC = BT // 512  # token chunks of 512 = 8
SCHUNKS = 17  # padded S = 2176 = 17 * 128
TB = 512  # attention t-block
NTB = T // TB  # 4 per batch
PAD_BIAS = -30.0

_cache: dict = {}


def _build_nc():
    import concourse.bass as bass
    import concourse.mybir as mybir
    import concourse.tile as tile
    from concourse import bacc

    fp32 = mybir.dt.float32
    fp32r = mybir.dt.float32r
    bf16 = mybir.dt.bfloat16

    nc = bacc.Bacc(
        "TRN2",
        target_bir_lowering=False,
        debug=False,
        enable_asserts=False,
        num_devices=NCORES,
    )

    # DRAM I/O (host-retiled for contiguous per-partition DMA lines)
    hti = nc.dram_tensor("hti", [NTC, 128, NKC, 512], bf16,
                         kind="ExternalInput").ap()
    wq = nc.dram_tensor("wq", [128, NKC, DC], bf16, kind="ExternalInput").ap()
    wk = nc.dram_tensor("wk", [128, NKC, DC], bf16, kind="ExternalInput").ap()
    wv = nc.dram_tensor("wv", [128, NKC, DC], bf16, kind="ExternalInput").ap()
    bq = nc.dram_tensor("bq", [DC, 1], fp32, kind="ExternalInput").ap()
    bk = nc.dram_tensor("bk", [DC, 1], fp32, kind="ExternalInput").ap()
    bv1 = nc.dram_tensor("bv1", [DC, 1], fp32, kind="ExternalInput").ap()
    wo = nc.dram_tensor("wo", [DC, D], bf16, kind="ExternalInput").ap()
    lkT = nc.dram_tensor("lkT", [B, DC, L], bf16, kind="ExternalInput").ap()
    lv = nc.dram_tensor("lv", [B, HPC, L, HD], bf16, kind="ExternalInput").ap()
    ebias0 = nc.dram_tensor("ebias0", [128, 1], fp32, kind="ExternalInput").ap()
    e2 = nc.dram_tensor("e2", [2, 128], fp32, kind="ExternalInput").ap()
    out = nc.dram_tensor("out", [BT, D], bf16, kind="ExternalOutput").ap()

    EXP = mybir.ActivationFunctionType.Exp

    with tile.TileContext(nc) as tc:
        with (
            tc.tile_pool(name="consts", bufs=1) as consts,
            tc.tile_pool(name="persist", bufs=1) as persist,
            tc.tile_pool(name="htiles", bufs=3) as htiles,
            tc.tile_pool(name="exps", bufs=4) as exps,
            tc.tile_pool(name="episb", bufs=2) as episb,
        ):
            # ---- constants (wq + first hidden group emitted first so the
            # q projection can start as early as possible) ----
            wq_sb = consts.tile([128, NKC, DC], bf16)
            wk_sb = consts.tile([128, NKC, DC], bf16)
            wv_sb = consts.tile([128, NKC, DC], bf16)
            nc.sync.dma_start(out=wq_sb, in_=wq)
            nc.gpsimd.dma_start(out=wk_sb, in_=wk)
            nc.gpsimd.dma_start(out=wv_sb, in_=wv)
            bq_sb = consts.tile([DC, 1], fp32)
            bk_sb = consts.tile([DC, 1], fp32)
            bv1_sb = consts.tile([DC, 1], fp32)
            wo_sb = consts.tile([DC, D], bf16)
            eb0_sb = consts.tile([128, 1], fp32)
            e2_sb = consts.tile([2, 128], fp32)
            e2_r = consts.tile([2, 128], fp32r)
            nc.sync.dma_start(out=bq_sb, in_=bq)
            nc.sync.dma_start(out=bk_sb, in_=bk)
            nc.sync.dma_start(out=bv1_sb, in_=bv1)
            nc.sync.dma_start(out=wo_sb, in_=wo)
            nc.sync.dma_start(out=e2_sb, in_=e2)
            nc.vector.tensor_copy(e2_r, e2_sb)
            nc.sync.dma_start(out=eb0_sb, in_=ebias0)

            # ---- persistent activations ----
            qT_sb = persist.tile([128, BT], bf16)  # [h0|h1 feats, global tok]
            kT_sb = persist.tile([128, B * SCHUNKS * 128], bf16)  # per b: 2176
            v_sb = persist.tile([128, B * HPC * SCHUNKS * 65], bf16)

            def k_off(b):
                return b * SCHUNKS * 128

            def v_off(b, h, c):
                return ((b * HPC + h) * SCHUNKS + c) * 65

            # latent / pad setup
            for b in range(B):
                nc.vector.memset(kT_sb[:, k_off(b) + L:k_off(b) + 128], 0.0)
                nc.sync.dma_start(out=kT_sb[:, k_off(b):k_off(b) + L],
                                  in_=lkT[b])
                for h in range(HPC):
                    nc.vector.memset(
                        v_sb[:, v_off(b, h, 0):v_off(b, h, 0) + 65], 0.0)
            # ones column for the Z fold (col 64 of every [128, 65] chunk)
            v_view = v_sb.rearrange("p (n c) -> p n c", c=65)
            nc.vector.memset(v_view[:, :, 64:65], 1.0)
            for b in range(B):
                for h in range(HPC):
                    nc.sync.dma_start(
                        out=v_sb[0:L, v_off(b, h, 0):v_off(b, h, 0) + HD],
                        in_=lv[b, h])

            # identity for PE transposes
            ident = consts.tile([128, 128], bf16)
            from concourse.masks import make_identity
            make_identity(nc, ident)

            # ---- psum pools (shared by qkv passes and attention) ----
            with (
                tc.tile_pool(name="scps", bufs=2, space="PSUM") as scps,
                tc.tile_pool(name="avps", bufs=4, space="PSUM") as avps,
            ):
                # ---- qkv projection closures for one 512-token chunk ----
                def qkv_closures(g):
                    t0g = g * 512
                    bb = t0g // T
                    c0 = (t0g - bb * T) // 128 + 1
                    hold = {}

                    def ht_a():
                        ht = htiles.tile([128, NKC, 512], bf16, tag="ht",
                                         name=f"ht_{g}")
                        nc.gpsimd.dma_start(out=ht[:, 0:4, :],
                                            in_=hti[g, :, 0:4, :])
                        hold["ht"] = ht

                    def ht_b():
                        nc.sync.dma_start(out=hold["ht"][:, 4:NKC, :],
                                          in_=hti[g, :, 4:NKC, :])

                    def mk_proj(key, w_sb):
                        def mk_part(k0, k1, tag=None):
                            def part():
                                if k0 == 0:
                                    ps = avps.tile([128, 512], fp32, tag="av",
                                                   name=f"{key}ps_{g}")
                                    hold[key] = ps
                                ps = hold[key]
                                for k in range(k0, k1):
                                    nc.tensor.matmul(
                                        ps, w_sb[:, k, :],
                                        hold["ht"][:, k, :],
                                        start=(k == 0), stop=(k == NKC - 1))
                            return part
                        return [mk_part(0, 2), mk_part(2, 4),
                                mk_part(4, 6), mk_part(6, 8)]

                    qp = mk_proj("q", wq_sb)
                    kp = mk_proj("k", wk_sb)
                    vp = mk_proj("v", wv_sb)

                    def q_fin():
                        nc.vector.tensor_scalar_add(
                            qT_sb[:, t0g:t0g + 512], hold["q"], bq_sb)

                    def k_fin():
                        koff = k_off(bb) + 128 + (t0g - bb * T)
                        nc.vector.tensor_scalar_add(
                            kT_sb[:, koff:koff + 512], hold["k"], bk_sb)

                    def v_fin():
                        vt = episb.tile([128, 512], bf16, tag="vt",
                                        name=f"vt_{g}")
                        nc.vector.tensor_scalar_add(vt, hold["v"], bv1_sb)
                        hold["vt"] = vt

                    def t_a():
                        tp = avps.tile([128, 512], bf16, tag="av",
                                       name=f"tp_{g}")
                        hold["tp"] = tp
                        for j in range(2):
                            nc.tensor.transpose(
                                tp[:, j * 128:(j + 1) * 128],
                                hold["vt"][:, j * 128:(j + 1) * 128], ident)

                    def t_b():
                        tp = hold["tp"]
                        for j in range(2, 4):
                            nc.tensor.transpose(
                                tp[:, j * 128:(j + 1) * 128],
                                hold["vt"][:, j * 128:(j + 1) * 128], ident)
                        # v_sb[:, (c0+m, h, d)] = tp[:, (m, h, d)]
                        dst = bass.AP(
                            tensor=v_sb.tensor,
                            offset=v_sb.offset + v_off(bb, 0, c0),
                            ap=[v_sb.ap[0], [65, 4], [SCHUNKS * 65, HPC],
                                [1, HD]])
                        srcv = tp.rearrange("p (m e) -> p m e", m=4)
                        src = bass.AP(
                            tensor=srcv.tensor, offset=srcv.offset,
                            ap=[srcv.ap[0], [128, 4], [64, HPC], [1, HD]])
                        nc.vector.tensor_copy(dst, src)

                    return ([ht_a, ht_b] + qp + [q_fin] + kp + [k_fin]
                            + vp + [v_fin, t_a, t_b])

                # ---- attention helpers ----
                def emit_epi_drain(st):
                    av0, av1 = st["av0"], st["av1"]
                    oz = episb.tile([128, 512], fp32, tag="oz",
                                    name=f"oz_{st['q0']}")
                    zh0 = episb.tile([1, 512], fp32, tag="zh0",
                                     name=f"zh0_{st['q0']}")
                    zh1 = episb.tile([1, 512], fp32, tag="zh1",
                                     name=f"zh1_{st['q0']}")
                    zr2 = episb.tile([2, 512], fp32, tag="zr2",
                                     name=f"zr2_{st['q0']}")
                    nc.vector.tensor_copy(zh0, av0[64:65, :])
                    nc.vector.tensor_copy(zh1, av1[64:65, :])
                    nc.gpsimd.dma_start(out=zr2[0:1, :], in_=zh0)
                    nc.gpsimd.dma_start(out=zr2[1:2, :], in_=zh1)
                    nc.vector.tensor_copy(oz[0:64, :], av0[0:64, :])
                    nc.vector.tensor_copy(oz[64:128, :], av1[0:64, :])
                    nc.vector.reciprocal_approx_fast(out=zr2, in_=zr2)
                    st["oz"], st["zr2"] = oz, zr2

                def emit_epi_zb(st):
                    zr2r = episb.tile([2, 512], fp32r, tag="zr2r",
                                      name=f"zr2r_{st['q0']}")
                    nc.vector.tensor_copy(zr2r, st["zr2"])
                    zb = avps.tile([128, 512], fp32, tag="av",
                                   name=f"zb_{st['q0']}")
                    nc.tensor.matmul(zb, e2_r, zr2r,
                                     start=True, stop=True)
                    ot = episb.tile([128, 512], bf16, tag="ot",
                                    name=f"ot_{st['q0']}")
                    nc.vector.tensor_mul(ot, st["oz"], zb)
                    st["ot"] = ot

                def mk_epi_out(st, j):
                    # out-proj for one 128-token block: 2 bf16 matmuls into
                    # two psum banks, one [128, 1024] bf16 copy, one DMA
                    def mm():
                        ot, q0 = st["ot"], st["q0"]
                        ops = []
                        for f in range(2):
                            op = avps.tile([128, 512], fp32, tag="av",
                                           name=f"op_{q0}_{j}_{f}")
                            nc.tensor.matmul(
                                op, ot[:, j * 128:(j + 1) * 128],
                                wo_sb[:, f * 512:(f + 1) * 512],
                                start=True, stop=True)
                            ops.append(op)
                        st[f"op{j}"] = ops

                    def wr():
                        q0 = st["q0"]
                        ops = st.pop(f"op{j}")
                        osb = episb.tile([128, D], bf16, tag="osb",
                                         name=f"osb_{q0}_{j}")
                        nc.vector.tensor_copy(osb[:, 0:512], ops[0])
                        nc.vector.tensor_copy(osb[:, 512:1024], ops[1])
                        r0 = q0 + j * 128
                        nc.sync.dma_start(out=out[r0:r0 + 128, :], in_=osb)
                    return [mm, wr]

                def emit_av(st, c):
                    b = st["b"]
                    stt, sp = c == 0, c == SCHUNKS - 1
                    ex = st["ex"].pop(c)
                    for h, av in ((0, st["av0"]), (1, st["av1"])):
                        vo = v_off(b, h, c)
                        eh = ex[:, h * 512:(h + 1) * 512]
                        nc.tensor.matmul(
                            av, v_sb[:, vo:vo + 65], eh,
                            start=stt, stop=sp)

                # ---- schedule: g0 inline; everything else paced through
                # two queues. gq holds qkv-group closures (tagged by group)
                # so chunks can force-drain exactly the groups whose k/v
                # they read -- emission order defines Tile dependencies, so
                # a chunk's scores must be emitted after its group's k_fin.
                from collections import deque
                g0 = qkv_closures(0)
                for cl in g0[:7]:  # ht loads, q projection, q_fin
                    cl()
                gq = deque()
                for g in range(1, NTC):
                    for cl in qkv_closures(g):
                        gq.append((g, cl))
                eq = deque()
                for _ in range(2):  # prefetch g1's ht DMAs
                    gq.popleft()[1]()

                def drain_groups(gmax):
                    while gq and gq[0][0] <= gmax:
                        gq.popleft()[1]()

                def pop_side(n):
                    for i in range(n):
                        if eq and (i % 2 == 0 or not gq):
                            eq.popleft()()
                        elif gq:
                            gq.popleft()[1]()
                        elif eq:
                            eq.popleft()()

                def mk_st(b, tb):
                    return {
                        "b": b, "q0": b * T + tb * TB,
                        "av0": avps.tile([65, 512], fp32, tag="av",
                                         name=f"av0_{b}_{tb}"),
                        "av1": avps.tile([65, 512], fp32, tag="av",
                                         name=f"av1_{b}_{tb}"),
                        "ex": {},
                    }

                def emit_chunk(st, c):
                    b, q0 = st["b"], st["q0"]
                    sc = scps.tile([128, 1024], fp32, tag="sc",
                                   name=f"sc_{b}_{q0}_{c}")
                    kc = k_off(b) + c * 128
                    nc.tensor.matmul(
                        sc[:, 0:512],
                        kT_sb[0:64, kc:kc + 128],
                        qT_sb[0:64, q0:q0 + TB],
                        start=True, stop=True)
                    nc.tensor.matmul(
                        sc[:, 512:1024],
                        kT_sb[64:128, kc:kc + 128],
                        qT_sb[64:128, q0:q0 + TB],
                        start=True, stop=True)
                    ex = exps.tile([128, 1024], bf16, tag="ex",
                                   name=f"ex_{b}_{q0}_{c}")
                    nc.scalar.activation(
                        ex, sc, EXP,
                        bias=(eb0_sb if c == 0 else 0.0), scale=1.0)
                    st["ex"][c] = ex

                for b in range(B):
                    for tb in range(NTB):
                        ti = b * NTB + tb
                        st = mk_st(b, tb)
                        if ti == 0:
                            # latent chunk needs only q: overlap its exp
                            # with g0's k/v projections
                            emit_chunk(st, 0)
                            for cl in g0[7:]:
                                cl()
                        else:
                            drain_groups(ti)  # this tb's q projection
                        for c in range(SCHUNKS):
                            if ti == 0 and c == 0:
                                continue
                            if c >= 1:
                                drain_groups(b * NTB + (c - 1) // 4)
                            emit_chunk(st, c)
                            if c >= 1:
                                emit_av(st, c - 1)
                            if tb == 0:
                                # race ahead on this batch's remaining
                                # projections (chunk c+1 group is needed
                                # at most 4 chunks out)
                                if c >= 1:
                                    pop_side(4)
                            elif ti == NTB - 1:
                                # pre-drain b1's projections before the
                                # batch switch
                                if c >= 1:
                                    pop_side(2)
                            else:
                                if c >= 1:
                                    pop_side(1)
                                if c in (5, 9, 13):
                                    pop_side(1)
                        emit_av(st, SCHUNKS - 1)
                        emit_epi_drain(st)
                        # queue this tb's epilogue (runs inside next tb);
                        # two no-op slots let the drain chain finish before zb
                        def mk_zb(s):
                            def go():
                                emit_epi_zb(s)
                            return go
                        noop = lambda: None
                        eq.append(noop)
                        eq.append(noop)
                        eq.append(mk_zb(st))
                        for j in range(4):
                            for cl in mk_epi_out(st, j):
                                eq.append(cl)
                # flush remaining side work (last epilogue + any stragglers)
                while gq:
                    gq.popleft()[1]()
                while eq:
                    eq.popleft()()

    nc.compile()
    return nc


def _get_nc():
    if "nc" not in _cache:
        _cache["nc"] = _build_nc()
    return _cache["nc"]


def _prep_inputs(hidden_states, decoder_latent, Wq, bq, Wk, bk, Wv, bv, Wo):
    """Build the 8 per-core input maps (host-side sharding/layout)."""
    hsD = np.ascontiguousarray(hidden_states.reshape(BT, D))
    # hti[g, p, k, t] = hidden[g*512+t, k*128+p]
    hti = np.ascontiguousarray(
        hsD.reshape(NTC, 512, NKC, 128).transpose(0, 3, 2, 1)).astype(BF16)
    lk = decoder_latent[..., :HD]  # [B, H, L, HD]
    lvf = decoder_latent[..., HD:]
    eb0 = np.full((128, 1), PAD_BIAS, np.float32)
    eb0[:L] = 0.0
    e2 = np.zeros((2, 128), np.float32)
    e2[0, 0:64] = 1.0
    e2[1, 64:128] = 1.0

    def retile_w(w):
        # [D, DC] -> [128, NKC, DC] with [p, k, :] = w[k*128+p, :]
        return np.ascontiguousarray(
            w.reshape(NKC, 128, DC).transpose(1, 0, 2)).astype(BF16)

    in_maps = []
    for c in range(NCORES):
        cols = slice(c * DC, (c + 1) * DC)
        h0, h1 = HPC * c, HPC * c + 1
        lkT_c = np.stack([
            np.concatenate([lk[b, h0].T, lk[b, h1].T], axis=0)
            for b in range(B)])  # [B, 128, L]
        in_maps.append({
            "hti": hti,
            "wq": retile_w(Wq[:, cols] * SCALE),
            "wk": retile_w(Wk[:, cols]),
            "wv": retile_w(Wv[:, cols]),
            "bq": (bq[cols] * SCALE).astype(np.float32).reshape(DC, 1),
            "bk": bk[cols].astype(np.float32).reshape(DC, 1),
            "bv1": bv[cols].astype(np.float32).reshape(DC, 1),
            "wo": Wo[cols, :].astype(BF16),
            "lkT": lkT_c.astype(BF16),
            "lv": lvf[:, h0:h1 + 1].astype(BF16),
            "ebias0": eb0,
            "e2": e2,
        })
    return in_maps


def _run(inputs, trace=False):
    from concourse.bass_utils import run_bass_kernel_spmd

    nc = _get_nc()
    in_maps = _prep_inputs(
        inputs["hidden_states"], inputs["decoder_latent"],
        inputs["Wq"], inputs["bq"], inputs["Wk"], inputs["bk"],
        inputs["Wv"], inputs["bv"], inputs["Wo"])
    res = run_bass_kernel_spmd(nc, in_maps, core_ids=list(range(NCORES)),
                               trace=trace)
    acc = np.zeros((BT, D), np.float64)
    for r in res.results:
        acc += r["out"].astype(np.float64)
    out = (acc + inputs["bo"].astype(np.float64)).astype(np.float32)
    return out.reshape(B, T, D), res


def _reference_fallback(hidden_states, decoder_latent, attention_mask,
                        Wq, bq, Wk, bk, Wv, bv, Wo, bo):
    """Exact numpy path, used only when attention_mask is non-zero (the
    problem spec fills it with zeros; the device kernel specializes on
    that)."""
    x = hidden_states.astype(np.float64)
    q = (x @ Wq + bq) * SCALE
    k = x @ Wk + bk
    v = x @ Wv + bv

    def heads(a):
        return a.reshape(B, T, H, HD).transpose(0, 2, 1, 3)

    q, k, v = heads(q), heads(k), heads(v)
    lk = decoder_latent[..., :HD].astype(np.float64)
    lv = decoder_latent[..., HD:].astype(np.float64)
    k = np.concatenate([lk, k], axis=2)
    v = np.concatenate([lv, v], axis=2)
    s = np.einsum("bhtd,bhsd->bhts", q, k) + attention_mask.astype(np.float64)
    s -= s.max(axis=-1, keepdims=True)
    p = np.exp(s)
    p /= p.sum(axis=-1, keepdims=True)
    o = np.einsum("bhts,bhsd->bhtd", p, v)
    o = o.transpose(0, 2, 1, 3).reshape(B, T, D)
    return (o @ Wo + bo).astype(np.float32)


def kernel(**inputs):
    inputs = {k: np.asarray(v) for k, v in inputs.items()}
    if np.any(inputs["attention_mask"]):
        return _reference_fallback(**inputs)
    out, _ = _run(inputs)
    return out
